# revision 6
# baseline (speedup 1.0000x reference)
"""Causal attention (B=2, S=2048, HID=2048, H=16, D=128) on 8 NeuronCores.

Sharding: tensor-parallel over heads — core c owns heads (2c, 2c+1).
Each core: projects Q/K/V for its heads (fp16 matmuls, fp32 PSUM accum),
applies rope (sign-permutation matmul + cos/sin DVE combine), computes
causal attention with transposed scores (k on partitions, q on free) so
softmax needs no transposes, then computes its partial contribution to
the output projection (contraction over its 256 hid columns of wo).
Host sums the 8 partial outputs.

Layouts (per core):
  xT   [2048 hid, 4096 (b*2048+s)] fp16   — x transposed, replicated
  wqT/wkT/wvT [2048 hid, 256 d] fp16      — weight slices, pre-transposed
  woT  [256 hid_c, 2048 e] fp16
  cose/sine [128 d, 2048 s] fp32          — rope tables (d row = d//2 freq)
  psgn [128, 128] fp16                    — rope pair-swap sign matrix (lhsT)
  tri  [128, 128] fp16                    — causal mask for diagonal tiles
Softmax uses exp without max subtraction: scores are ~N(0,1) after the
1/sqrt(D) scale (max |score| ~ 7), safe in fp32/fp16.
"""
import math
import sys
import types

import numpy as np

B, S, HID, H = 2, 2048, 2048, 16
D = 128
NCORES = 8
HPC = H // NCORES            # heads per core
DC = HPC * D                 # hid slice per core
SC = 512                     # seq chunk
NQC = S // SC                # chunks per batch
NHT = HID // 128             # hid tiles
F16 = np.float16


def _register_ntff_hook():
    """The agent image's antenv lacks axon_hooks; recreate it so
    run_bass_kernel_spmd(trace=True) can capture NTFF profiles."""
    try:
        from antenv.axon_hooks import get_axon_ntff_profile_hook  # noqa: F401
        return
    except ImportError:
        pass
    try:
        import antenv
        from trn_agent_boot.trn_boot import _ntff_profile_via_ctypes
        mod = types.ModuleType("antenv.axon_hooks")
        _hook = [None]
        mod.set_axon_ntff_profile_hook = lambda h: _hook.__setitem__(0, h)
        mod.get_axon_ntff_profile_hook = lambda: _hook[0]
        sys.modules["antenv.axon_hooks"] = mod
        antenv.axon_hooks = mod
        mod.set_axon_ntff_profile_hook(
            _ntff_profile_via_ctypes("/opt/axon/libaxon_pjrt.so"))
    except Exception:
        pass


_CACHE = {}
LAST_RESULT = None


def _build():
    import concourse.bass as bass  # noqa: F401
    import concourse.mybir as mybir
    import concourse.tile as tile
    from concourse import bacc

    f16 = mybir.dt.float16
    f32 = mybir.dt.float32
    EXP = mybir.ActivationFunctionType.Exp
    ISCALE = float(1.0 / math.sqrt(D))

    nc = bacc.Bacc("TRN2", target_bir_lowering=False, debug=False,
                   num_devices=NCORES)

    xT = nc.dram_tensor("xT", [HID, B * S], f16, kind="ExternalInput").ap()
    # wq is split so the first projection group only waits on half the bytes
    wqTa = nc.dram_tensor("wqTa", [HID, 128], f16, kind="ExternalInput").ap()
    wqTb = nc.dram_tensor("wqTb", [HID, 128], f16, kind="ExternalInput").ap()
    wkT = nc.dram_tensor("wkT", [HID, DC], f16, kind="ExternalInput").ap()
    wvT = nc.dram_tensor("wvT", [HID, DC], f16, kind="ExternalInput").ap()
    woT = nc.dram_tensor("woT", [DC, HID], f16, kind="ExternalInput").ap()
    cose = nc.dram_tensor("cose", [128, S], f32, kind="ExternalInput").ap()
    sine = nc.dram_tensor("sine", [128, S], f32, kind="ExternalInput").ap()
    psgn = nc.dram_tensor("psgn", [128, 128], f16, kind="ExternalInput").ap()
    tri = nc.dram_tensor("tri", [128, 128], f16, kind="ExternalInput").ap()
    ident = nc.dram_tensor("ident", [128, 128], f16, kind="ExternalInput").ap()
    out = nc.dram_tensor("out", [B * S, HID], f16, kind="ExternalOutput").ap()

    with tile.TileContext(nc) as tc:
        with tc.tile_pool(name="consts", bufs=1) as consts, \
             tc.tile_pool(name="kv", bufs=1) as kv, \
             tc.tile_pool(name="xtp", bufs=2) as xtp, \
             tc.tile_pool(name="qdp", bufs=2) as qdp, \
             tc.tile_pool(name="ropep", bufs=4) as ropep, \
             tc.tile_pool(name="ptp", bufs=6) as ptp, \
             tc.tile_pool(name="psump", bufs=2) as psump, \
             tc.tile_pool(name="onp", bufs=2) as onp, \
             tc.tile_pool(name="bcp", bufs=2) as bcp, \
             tc.tile_pool(name="stgp", bufs=2) as stgp, \
             tc.tile_pool(name="pstream", bufs=2, space="PSUM") as pstream, \
             tc.tile_pool(name="pacc", bufs=2, space="PSUM") as pacc, \
             tc.tile_pool(name="pproj", bufs=3, space="PSUM") as pproj, \
             tc.tile_pool(name="lps", bufs=1, space="PSUM") as lps:

            # ---- constants ----
            # small psgn/tri first so PE warmup can start; the big weight
            # loads are deferred until after the first x chunk slice is on
            # the queue (load_late_consts) so the first projection matmuls
            # wait on as few bytes as possible.
            psgn_sb = consts.tile([128, 128], f16)
            tri_sb = consts.tile([128, 128], f16)
            ident_sb = consts.tile([128, 128], f16)
            nc.sync.dma_start(out=psgn_sb, in_=psgn[:, :])
            nc.sync.dma_start(out=tri_sb, in_=tri[:, :])
            nc.sync.dma_start(out=ident_sb, in_=ident[:, :])
            # warm up the PE (HAM clock gate) while the first DMAs land
            warm_ps = pstream.tile([128, 128], f32, tag="st")
            for _ in range(48):
                nc.tensor.matmul(warm_ps, lhsT=psgn_sb, rhs=tri_sb,
                                 start=True, stop=True)
            wq_sb = consts.tile([128, NHT, DC], f16)
            wk_sb = consts.tile([128, NHT, DC], f16)
            wv_sb = consts.tile([128, NHT, DC], f16)
            cos_sb = consts.tile([128, S], f32)
            sin_sb = consts.tile([128, S], f32)
            ones_mat = consts.tile([128, 128], f16)
            nc.vector.memset(ones_mat, 1.0)
            wo_sb = consts.tile([128, HPC, HID], f16)

            def load_late_consts():
                # issued after the first chunk's xT halves; split across the
                # sync and gpsimd queues, ordered by first-consumption time.
                # q dt=0 needs only wqTa (0.5MB) after xta/xtb.
                nc.sync.dma_start(
                    out=wq_sb[:, :, 0:128],
                    in_=wqTa.rearrange("(ht p) d -> p ht d", p=128))
                nc.gpsimd.dma_start(
                    out=wq_sb[:, :, 128:256],
                    in_=wqTb.rearrange("(ht p) d -> p ht d", p=128))
                nc.sync.dma_start(
                    out=wk_sb, in_=wkT.rearrange("(ht p) d -> p ht d", p=128))
                nc.gpsimd.dma_start(
                    out=wv_sb, in_=wvT.rearrange("(ht p) d -> p ht d", p=128))
                nc.sync.dma_start(out=cos_sb, in_=cose[:, :])
                nc.gpsimd.dma_start(out=sin_sb, in_=sine[:, :])
                nc.gpsimd.dma_start(
                    out=wo_sb, in_=woT.rearrange("(dt p) e -> p dt e", p=128))

            # ---- persistent K/V for the core's heads ----
            kd_sb = kv.tile([128, HPC, B * S], f16)     # [d, head, b*2048+s]
            v_sb = kv.tile([128, B * S // 128, DC], f16)  # [s%128, stile, (h,d)]

            def wo_phase(on_sb, s0g, stage, sts=(0, 1, 2, 3),
                         emit_dma=False):
                # ---------- output projection for a finished chunk ----------
                # results staged in SBUF (fp16 partials) and shipped with a
                # single DMA per chunk; sub-phases are interleaved into the
                # next chunk's attention so the DVE drain overlaps the
                # k-loops instead of stalling the projection
                for st in sts:
                    for ec in range(HID // 512):
                        fin_ps = pproj.tile([128, 512], f32, tag="pj")
                        for dt in range(HPC):
                            nc.tensor.matmul(
                                fin_ps,
                                lhsT=on_sb[:, dt, st * 128:(st + 1) * 128],
                                rhs=wo_sb[:, dt, ec * 512:(ec + 1) * 512],
                                start=(dt == 0), stop=(dt == HPC - 1))
                        nc.vector.tensor_copy(
                            out=stage[:, st, ec * 512:(ec + 1) * 512],
                            in_=fin_ps)
                if emit_dma:
                    nc.sync.dma_start(
                        out=out[s0g:s0g + SC, :].rearrange(
                            "(st p) e -> p st e", p=128),
                        in_=stage)

            pending_wo = None  # (on_sb, s0g, stage) of the previous chunk
            for b in range(B):
                for qc in range(NQC):
                    s0g = b * S + qc * SC
                    q0 = qc * SC

                    # ---------- projection of this chunk ----------
                    # two separate half-tiles so dependency tracking lets
                    # ht 0-7 matmuls start before the second half lands;
                    # first chunk puts that half on the sync queue, right
                    # behind wq
                    half = NHT // 2
                    xta = xtp.tile([128, half, SC], f16, tag="xta")
                    xtb = xtp.tile([128, half, SC], f16, tag="xtb")
                    dma_a = nc.sync.dma_start if s0g == 0 \
                        else nc.gpsimd.dma_start
                    dma_a(out=xta,
                          in_=xT[:half * 128, s0g:s0g + SC].rearrange(
                              "(ht p) s -> p ht s", p=128))
                    nc.gpsimd.dma_start(
                        out=xtb,
                        in_=xT[half * 128:, s0g:s0g + SC].rearrange(
                            "(ht p) s -> p ht s", p=128))
                    if s0g == 0:
                        load_late_consts()

                    def xt_sl(ht, cols=slice(None)):
                        t = xta if ht < half else xtb
                        return t[:, ht % half, cols]

                    qd_c = qdp.tile([128, HPC, SC], f16, tag="qd")

                    def emit_rope(acc, qraw, dest):
                        # dest = acc*cos + (psgn.T@acc)*sin — the u matmul
                        # is emitted one projection group late so the PE
                        # doesn't stall on the qraw ACT copy
                        u_ps = pstream.tile([128, SC], f32, tag="st")
                        nc.tensor.matmul(u_ps, lhsT=psgn_sb, rhs=qraw,
                                         start=True, stop=True)
                        t0 = ropep.tile([128, SC], f16, tag="t0")
                        nc.vector.tensor_mul(t0, acc, cos_sb[:, q0:q0 + SC])
                        t1 = ropep.tile([128, SC], f16, tag="t1")
                        nc.vector.tensor_mul(t1, u_ps, sin_sb[:, q0:q0 + SC])
                        nc.vector.tensor_add(dest, t0, t1)

                    rope_pending = None
                    for which in ("q", "k"):
                        w_sb = wq_sb if which == "q" else wk_sb
                        for dt in range(HPC):
                            acc = pproj.tile([128, SC], f32, tag="pj")
                            for ht in range(NHT):
                                nc.tensor.matmul(
                                    acc,
                                    lhsT=w_sb[:, ht, dt * 128:(dt + 1) * 128],
                                    rhs=xt_sl(ht),
                                    start=(ht == 0), stop=(ht == NHT - 1))
                            qraw = ropep.tile([128, SC], f16, tag="qraw")
                            nc.scalar.copy(out=qraw, in_=acc)
                            if rope_pending is not None:
                                emit_rope(*rope_pending)
                            if which == "q":
                                dest = qd_c[:, dt, :]
                            else:
                                dest = kd_sb[:, dt, s0g:s0g + SC]
                            rope_pending = (acc, qraw, dest)
                    for st in range(SC // 128):
                        vacc = pproj.tile([128, DC], f32, tag="pj")
                        for ht in range(NHT):
                            nc.tensor.matmul(
                                vacc,
                                lhsT=xt_sl(ht, slice(st * 128,
                                                     (st + 1) * 128)),
                                rhs=wv_sb[:, ht, :],
                                start=(ht == 0), stop=(ht == NHT - 1))
                        if rope_pending is not None:
                            emit_rope(*rope_pending)
                            rope_pending = None
                        nc.vector.tensor_copy(
                            out=v_sb[:, (s0g // 128) + st, :], in_=vacc)

                    # previous chunk's output projection: half here (PE work
                    # covering the rope DVE chain), half between the heads
                    # (so the fin-copy drain overlaps the h1 k-loop)
                    if pending_wo is not None:
                        wo_phase(*pending_wo, sts=(0, 1))

                    # ---------- attention for this chunk ----------
                    on_sb = onp.tile([128, HPC, SC], f16, tag="on")
                    stage = stgp.tile([128, SC // 128, HID], f16, tag="stg")
                    for h in range(HPC):
                        oT = pacc.tile([128, SC], f32, tag="acc")
                        # column sums of probs, broadcast to all 128
                        # partitions by an all-ones stationary matrix; prob
                        # tiles are pre-summed in groups of 4 on GpSimd so
                        # the PE streams 4x fewer ones-matmul columns
                        lbc_ps = lps.tile([128, SC], f32, tag="l")
                        nkt = qc * 4 + 4
                        ngrp = nkt // 4

                        def emit_probs(kt):
                            # scores matmul + exp; on the diagonal tile a
                            # second tiny matmul accumulates a -60000
                            # upper-triangle bias (tri_sb.T @ I) so exp
                            # gives exact causal zeros — no vector-engine
                            # masking in the dependency chain
                            col0 = max(0, 128 * kt - q0)
                            diag = kt >= qc * 4
                            sp = pstream.tile([128, SC], f32, tag="st")
                            nc.tensor.matmul(
                                sp[:, col0:],
                                lhsT=kd_sb[:, h,
                                           b * S + kt * 128:
                                           b * S + (kt + 1) * 128],
                                rhs=qd_c[:, h, col0:],
                                start=True, stop=not diag)
                            if diag:
                                j = 128 * (kt - qc * 4)
                                nc.tensor.matmul(
                                    sp[:, j:j + 128], lhsT=tri_sb,
                                    rhs=ident_sb, start=False, stop=True)
                            pT = ptp.tile([128, SC], f16, tag="pt")
                            nc.scalar.activation(out=pT[:, col0:],
                                                 in_=sp[:, col0:],
                                                 func=EXP, scale=ISCALE)
                            return pT

                        # software-pipelined (depth 2): scores for kt+1/kt+2
                        # are emitted before the l/PV matmuls of kt, so the
                        # PE has work while the exp for kt runs on ACT
                        pts = [emit_probs(kt) for kt in range(min(2, nkt))]
                        acc_g = None
                        for kt in range(nkt):
                            col0 = max(0, 128 * kt - q0)
                            if kt + 2 < nkt:
                                pts.append(emit_probs(kt + 2))
                            pT_cur = pts.pop(0)
                            g, j = divmod(kt, 4)
                            if j == 0:
                                acc_g = psump.tile([128, SC], f16, tag="ps")
                                nc.gpsimd.tensor_copy(
                                    out=acc_g[:, col0:], in_=pT_cur[:, col0:])
                            else:
                                nc.gpsimd.tensor_add(
                                    acc_g[:, col0:], acc_g[:, col0:],
                                    pT_cur[:, col0:])
                            if j == 3:
                                gcol0 = max(0, 128 * 4 * g - q0)
                                nc.tensor.matmul(
                                    lbc_ps[:, gcol0:], lhsT=ones_mat,
                                    rhs=acc_g[:, gcol0:],
                                    start=(g == 0), stop=(g == ngrp - 1))
                            nc.tensor.matmul(
                                oT[:, col0:],
                                lhsT=v_sb[:, b * (S // 128) + kt,
                                          h * 128:(h + 1) * 128],
                                rhs=pT_cur[:, col0:],
                                start=(kt == 0), stop=(kt == nkt - 1))
                        # free the l PSUM bank promptly via an ACT copy
                        # (the DVE queue may be clogged by fin copies),
                        # then reciprocal + normalize from SBUF
                        l_sb = bcp.tile([128, SC], f32, tag="lsb")
                        nc.scalar.copy(out=l_sb, in_=lbc_ps)
                        rbc = bcp.tile([128, SC], f32, tag="rbc")
                        nc.vector.reciprocal_approx_fast(out=rbc, in_=l_sb)
                        nc.vector.tensor_mul(on_sb[:, h, :], oT, rbc)
                        if h == 0 and pending_wo is not None:
                            wo_phase(*pending_wo, sts=(2, 3), emit_dma=True)
                            pending_wo = None

                    pending_wo = (on_sb, s0g, stage)
            wo_phase(*pending_wo, emit_dma=True)
    nc.compile()
    return nc


def _prep_inputs(x, freqs_cis, wq, wk, wv, wo):
    x = np.asarray(x, dtype=np.float32)
    freqs = np.asarray(freqs_cis, dtype=np.float32)
    wq = np.asarray(wq, dtype=np.float32)
    wk = np.asarray(wk, dtype=np.float32)
    wv = np.asarray(wv, dtype=np.float32)
    wo = np.asarray(wo, dtype=np.float32)

    xT = x.reshape(B * S, HID).T.astype(F16, order="C")
    cos_e = np.ascontiguousarray(np.repeat(freqs[:, :, 0].T, 2, axis=0),
                                 dtype=np.float32)
    sin_e = np.ascontiguousarray(np.repeat(freqs[:, :, 1].T, 2, axis=0),
                                 dtype=np.float32)
    psgn = np.zeros((128, 128), F16)
    idx = np.arange(64)
    psgn[2 * idx, 2 * idx + 1] = 1.0
    psgn[2 * idx + 1, 2 * idx] = -1.0
    # causal bias, passed pre-transposed for use as matmul lhsT:
    # bias[kp, qf] = -60000 where kp > qf (future key), else 0
    kp = np.arange(128)[:, None]
    qf = np.arange(128)[None, :]
    tri = np.ascontiguousarray(np.where(kp > qf, -60000.0, 0.0).T
                               ).astype(F16)
    ident = np.eye(128, dtype=F16)

    in_maps = []
    for c in range(NCORES):
        sl = slice(DC * c, DC * (c + 1))
        wqT = wq[sl, :].T.astype(F16, order="C")
        in_maps.append({
            "xT": xT,
            "wqTa": np.ascontiguousarray(wqT[:, :128]),
            "wqTb": np.ascontiguousarray(wqT[:, 128:]),
            "wkT": wk[sl, :].T.astype(F16, order="C"),
            "wvT": wv[sl, :].T.astype(F16, order="C"),
            "woT": wo[:, sl].T.astype(F16, order="C"),
            "cose": cos_e,
            "sine": sin_e,
            "psgn": psgn,
            "tri": tri,
            "ident": ident,
        })
    return in_maps


def kernel(x, freqs_cis, wq, wk, wv, wo):
    global LAST_RESULT
    _register_ntff_hook()
    from concourse import bass_utils

    if "nc" not in _CACHE:
        _CACHE["nc"] = _build()
    nc = _CACHE["nc"]

    in_maps = _prep_inputs(x, freqs_cis, wq, wk, wv, wo)
    res = bass_utils.run_bass_kernel_spmd(
        nc, in_maps, core_ids=list(range(NCORES)))
    LAST_RESULT = res
    acc = np.zeros((B * S, HID), np.float64)
    for r in res.results:
        acc += r["out"].astype(np.float64)
    return acc.reshape(B, S, HID).astype(np.float32)



# revision 15
# speedup vs baseline: 1.2856x; 1.2856x over previous
"""Causal attention (B=2, S=2048, HID=2048, H=16, D=128) on 8 NeuronCores.

Sharding: tensor-parallel over heads — core c owns heads (2c, 2c+1).
Each core: projects Q/K/V for its heads (fp16 matmuls, fp32 PSUM accum),
applies rope (sign-permutation matmul + cos/sin DVE combine), computes
causal attention with transposed scores (k on partitions, q on free) so
softmax needs no transposes, then computes its partial contribution to
the output projection (contraction over its 256 hid columns of wo).
Host sums the 8 fp16 partial outputs.

Layouts (per core):
  xT   [2048 hid, 4096 (b*2048+s)] fp16   — x transposed, replicated
  wqTa/wqTb [2048 hid, 128 d] fp16        — wq slice halves (dt=0 / dt=1),
                                            rope-permuted columns
  wkT  [2048 hid, 256 d] fp16
  wvT  [2048 hid, 256 d] fp16
  woT  [256 hid_c, 2048 e] fp16
  cose/sine [128 d, 2048 s] fp32          — rope tables (d row = d//2 freq)
  tri  [128, 128] fp16                    — causal mask for diagonal tiles
Softmax uses exp without max subtraction: scores are ~N(0,1) after the
1/sqrt(D) scale (max |score| ~ 7), safe in fp32/fp16.
"""
import math
import sys
import types

import numpy as np

B, S, HID, H = 2, 2048, 2048, 16
D = 128
NCORES = 8
HPC = H // NCORES            # heads per core
DC = HPC * D                 # hid slice per core
SC = 512                     # seq chunk
NQC = S // SC                # chunks per batch
NHT = HID // 128             # hid tiles
F16 = np.float16


def _register_ntff_hook():
    """The agent image's antenv lacks axon_hooks; recreate it so
    run_bass_kernel_spmd(trace=True) can capture NTFF profiles."""
    try:
        from antenv.axon_hooks import get_axon_ntff_profile_hook  # noqa: F401
        return
    except ImportError:
        pass
    try:
        import antenv
        from trn_agent_boot.trn_boot import _ntff_profile_via_ctypes
        mod = types.ModuleType("antenv.axon_hooks")
        _hook = [None]
        mod.set_axon_ntff_profile_hook = lambda h: _hook.__setitem__(0, h)
        mod.get_axon_ntff_profile_hook = lambda: _hook[0]
        sys.modules["antenv.axon_hooks"] = mod
        antenv.axon_hooks = mod
        mod.set_axon_ntff_profile_hook(
            _ntff_profile_via_ctypes("/opt/axon/libaxon_pjrt.so"))
    except Exception:
        pass


_CACHE = {}
LAST_RESULT = None


def _build():
    import concourse.bass as bass  # noqa: F401
    import concourse.mybir as mybir
    import concourse.tile as tile
    from concourse import bacc

    f16 = mybir.dt.float16
    f32 = mybir.dt.float32
    EXP = mybir.ActivationFunctionType.Exp
    ISCALE = float(1.0 / math.sqrt(D))

    nc = bacc.Bacc("TRN2", target_bir_lowering=False, debug=False,
                   num_devices=NCORES)

    xT = nc.dram_tensor("xT", [HID, B * S], f16, kind="ExternalInput").ap()
    # wq split so the first projection group waits on only half the bytes
    wqTa = nc.dram_tensor("wqTa", [HID, 128], f16, kind="ExternalInput").ap()
    wqTb = nc.dram_tensor("wqTb", [HID, 128], f16, kind="ExternalInput").ap()
    wkT = nc.dram_tensor("wkT", [HID, DC], f16, kind="ExternalInput").ap()
    wvT = nc.dram_tensor("wvT", [HID, DC], f16, kind="ExternalInput").ap()
    woT = nc.dram_tensor("woT", [DC, HID], f16, kind="ExternalInput").ap()
    cose = nc.dram_tensor("cose", [128, S], f32, kind="ExternalInput").ap()
    sine = nc.dram_tensor("sine", [128, S], f32, kind="ExternalInput").ap()
    psgn = nc.dram_tensor("psgn", [128, 128], f16, kind="ExternalInput").ap()
    tri = nc.dram_tensor("tri", [128, 128], f16, kind="ExternalInput").ap()
    ident = nc.dram_tensor("ident", [128, 128], f16, kind="ExternalInput").ap()
    out = nc.dram_tensor("out", [B * S, HID], f16, kind="ExternalOutput").ap()

    with tile.TileContext(nc) as tc:
        with tc.tile_pool(name="consts", bufs=1) as consts, \
             tc.tile_pool(name="kv", bufs=1) as kv, \
             tc.tile_pool(name="xtp", bufs=2) as xtp, \
             tc.tile_pool(name="qdp", bufs=2) as qdp, \
             tc.tile_pool(name="ropep", bufs=4) as ropep, \
             tc.tile_pool(name="ptp", bufs=4) as ptp, \
             tc.tile_pool(name="onp", bufs=2) as onp, \
             tc.tile_pool(name="bcp", bufs=2) as bcp, \
             tc.tile_pool(name="stgp", bufs=2) as stgp, \
             tc.tile_pool(name="pstream", bufs=2, space="PSUM") as pstream, \
             tc.tile_pool(name="pacc", bufs=2, space="PSUM") as pacc, \
             tc.tile_pool(name="pproj", bufs=3, space="PSUM") as pproj, \
             tc.tile_pool(name="lps", bufs=1, space="PSUM") as lps:

            # ---- constants ----
            # small psgn/tri first so PE warmup can start; everything else
            # streams on the sync queue behind the first chunk's critical
            # loads (xta+wqa), ordered by first-consumption time.  The
            # gpsimd queue is kept clear for x chunk streaming.
            psgn_sb = consts.tile([128, 128], f16)
            tri_sb = consts.tile([128, 128], f16)
            ident_sb = consts.tile([128, 128], f16)
            nc.sync.dma_start(out=psgn_sb, in_=psgn[:, :])
            nc.sync.dma_start(out=tri_sb, in_=tri[:, :])
            nc.sync.dma_start(out=ident_sb, in_=ident[:, :])
            # warm up the PE (HAM clock gate) while the first DMAs land
            warm_ps = pstream.tile([128, 128], f32, tag="st")
            for _ in range(48):
                nc.tensor.matmul(warm_ps, lhsT=psgn_sb, rhs=tri_sb,
                                 start=True, stop=True)
            wq_sb = consts.tile([128, NHT, DC], f16)
            wk_sb = consts.tile([128, NHT, DC], f16)
            wv_sb = consts.tile([128, NHT, DC], f16)
            cos_sb = consts.tile([128, S], f32)
            sin_sb = consts.tile([128, S], f32)
            ones_mat = consts.tile([128, 128], f16)
            nc.vector.memset(ones_mat, 1.0)
            wo_sb = consts.tile([128, HPC, HID], f16)

            def load_late_consts():
                # issued after the first chunk's xta; all on the sync queue
                # so they cannot overtake the critical first loads
                nc.sync.dma_start(
                    out=wq_sb[:, :, 0:128],
                    in_=wqTa.rearrange("(ht p) d -> p ht d", p=128))
                nc.sync.dma_start(
                    out=wq_sb[:, :, 128:256],
                    in_=wqTb.rearrange("(ht p) d -> p ht d", p=128))
                nc.sync.dma_start(out=cos_sb, in_=cose[:, :])
                nc.sync.dma_start(
                    out=wk_sb, in_=wkT.rearrange("(ht p) d -> p ht d", p=128))
                nc.sync.dma_start(out=sin_sb, in_=sine[:, :])
                nc.sync.dma_start(
                    out=wv_sb, in_=wvT.rearrange("(ht p) d -> p ht d", p=128))
                nc.sync.dma_start(
                    out=wo_sb, in_=woT.rearrange("(dt p) e -> p dt e", p=128))

            # ---- persistent K/V for the core's heads ----
            kd_sb = kv.tile([128, HPC, B * S], f16)     # [d, head, b*2048+s]
            v_sb = kv.tile([128, B * S // 128, DC], f16)  # [s%128, stile, (h,d)]

            def wo_phase(on_sb, s0g, stage, sts=(0, 1, 2, 3),
                         emit_dma=False, dma_per_st=False):
                # ---------- output projection for a finished chunk ----------
                # results staged in SBUF (fp16 partials) and shipped with a
                # single DMA per chunk (or per st-row for the final chunk so
                # the last transfer isn't one big serial tail); sub-phases
                # are interleaved into the next chunk's attention
                for st in sts:
                    for ec in range(HID // 512):
                        fin_ps = pproj.tile([128, 512], f32, tag="pj")
                        for dt in range(HPC):
                            nc.tensor.matmul(
                                fin_ps,
                                lhsT=on_sb[:, dt, st * 128:(st + 1) * 128],
                                rhs=wo_sb[:, dt, ec * 512:(ec + 1) * 512],
                                start=(dt == 0), stop=(dt == HPC - 1))
                        nc.vector.tensor_copy(
                            out=stage[:, st, ec * 512:(ec + 1) * 512],
                            in_=fin_ps)
                    if dma_per_st:
                        nc.sync.dma_start(
                            out=out[s0g + st * 128:s0g + (st + 1) * 128, :],
                            in_=stage[:, st, :])
                if emit_dma and not dma_per_st:
                    nc.sync.dma_start(
                        out=out[s0g:s0g + SC, :].rearrange(
                            "(st p) e -> p st e", p=128),
                        in_=stage)

            pending_wo = None  # (on_sb, s0g, stage) of the previous chunk
            for b in range(B):
                for qc in range(NQC):
                    s0g = b * S + qc * SC
                    q0 = qc * SC
                    last_chunk = (b == B - 1) and (qc == NQC - 1)

                    # ---------- projection of this chunk ----------
                    # two separate half-tiles so dependency tracking lets
                    # ht 0-7 matmuls start before the second half lands;
                    # first chunk puts them on the sync queue ahead of the
                    # weight stream
                    half = NHT // 2
                    xta = xtp.tile([128, half, SC], f16, tag="xta")
                    xtb = xtp.tile([128, half, SC], f16, tag="xtb")
                    dma_a = nc.sync.dma_start if s0g == 0 \
                        else nc.gpsimd.dma_start
                    dma_a(out=xta,
                          in_=xT[:half * 128, s0g:s0g + SC].rearrange(
                              "(ht p) s -> p ht s", p=128))
                    nc.gpsimd.dma_start(
                        out=xtb,
                        in_=xT[half * 128:, s0g:s0g + SC].rearrange(
                            "(ht p) s -> p ht s", p=128))
                    if s0g == 0:
                        load_late_consts()

                    def xt_sl(ht, cols=slice(None)):
                        t = xta if ht < half else xtb
                        return t[:, ht % half, cols]

                    qd_c = qdp.tile([128, HPC, SC], f16, tag="qd")

                    def emit_rope(acc, qraw, dest):
                        # dest = acc*cos + (psgn.T@acc)*sin — the u matmul
                        # is emitted one projection group late so the PE
                        # doesn't stall on the qraw ACT copy
                        u_ps = pstream.tile([128, SC], f32, tag="st")
                        nc.tensor.matmul(u_ps, lhsT=psgn_sb, rhs=qraw,
                                         start=True, stop=True)
                        t0 = ropep.tile([128, SC], f16, tag="t0")
                        nc.vector.tensor_mul(t0, acc, cos_sb[:, q0:q0 + SC])
                        t1 = ropep.tile([128, SC], f16, tag="t1")
                        nc.vector.tensor_mul(t1, u_ps, sin_sb[:, q0:q0 + SC])
                        nc.vector.tensor_add(dest, t0, t1)

                    rope_pending = None
                    for which in ("q", "k"):
                        w_sb = wq_sb if which == "q" else wk_sb
                        for dt in range(HPC):
                            acc = pproj.tile([128, SC], f32, tag="pj")
                            for ht in range(NHT):
                                nc.tensor.matmul(
                                    acc,
                                    lhsT=w_sb[:, ht, dt * 128:(dt + 1) * 128],
                                    rhs=xt_sl(ht),
                                    start=(ht == 0), stop=(ht == NHT - 1))
                            qraw = ropep.tile([128, SC], f16, tag="qraw")
                            nc.scalar.copy(out=qraw, in_=acc)
                            if rope_pending is not None:
                                emit_rope(*rope_pending)
                            if which == "q":
                                dest = qd_c[:, dt, :]
                            else:
                                dest = kd_sb[:, dt, s0g:s0g + SC]
                            rope_pending = (acc, qraw, dest)
                    for st in range(SC // 128):
                        vacc = pproj.tile([128, DC], f32, tag="pj")
                        for ht in range(NHT):
                            nc.tensor.matmul(
                                vacc,
                                lhsT=xt_sl(ht, slice(st * 128,
                                                     (st + 1) * 128)),
                                rhs=wv_sb[:, ht, :],
                                start=(ht == 0), stop=(ht == NHT - 1))
                        if rope_pending is not None:
                            emit_rope(*rope_pending)
                            rope_pending = None
                        nc.vector.tensor_copy(
                            out=v_sb[:, (s0g // 128) + st, :], in_=vacc)

                    # previous chunk's output projection: half here (PE work
                    # covering the rope DVE chain), half between the heads
                    # (so the fin-copy drain overlaps the h1 k-loop)
                    if pending_wo is not None:
                        wo_phase(*pending_wo, sts=(0, 1))

                    # ---------- attention for this chunk ----------
                    on_sb = onp.tile([128, HPC, SC], f16, tag="on")
                    stage = stgp.tile([128, SC // 128, HID], f16, tag="stg")
                    for h in range(HPC):
                        oT = pacc.tile([128, SC], f32, tag="acc")
                        # column sums of probs, broadcast to all 128
                        # partitions by an all-ones stationary matrix
                        lbc_ps = lps.tile([128, SC], f32, tag="l")
                        nkt = qc * 4 + 4

                        def emit_probs(kt):
                            # scores matmul + exp; on the diagonal tile a
                            # second tiny matmul accumulates a -60000
                            # upper-triangle bias (tri_sb.T @ I) so exp
                            # gives exact causal zeros — no vector-engine
                            # masking in the dependency chain
                            col0 = max(0, 128 * kt - q0)
                            diag = kt >= qc * 4
                            sp = pstream.tile([128, SC], f32, tag="st")
                            nc.tensor.matmul(
                                sp[:, col0:],
                                lhsT=kd_sb[:, h,
                                           b * S + kt * 128:
                                           b * S + (kt + 1) * 128],
                                rhs=qd_c[:, h, col0:],
                                start=True, stop=not diag)
                            if diag:
                                j = 128 * (kt - qc * 4)
                                nc.tensor.matmul(
                                    sp[:, j:j + 128], lhsT=tri_sb,
                                    rhs=ident_sb, start=False, stop=True)
                            pT = ptp.tile([128, SC], f16, tag="pt")
                            nc.scalar.activation(out=pT[:, col0:],
                                                 in_=sp[:, col0:],
                                                 func=EXP, scale=ISCALE)
                            return pT

                        # software-pipelined (depth 2): scores for kt+1/kt+2
                        # are emitted before the l/PV matmuls of kt, so the
                        # PE has work while the exp for kt runs on ACT
                        pts = [emit_probs(kt) for kt in range(min(2, nkt))]
                        for kt in range(nkt):
                            col0 = max(0, 128 * kt - q0)
                            if kt + 2 < nkt:
                                pts.append(emit_probs(kt + 2))
                            pT_cur = pts.pop(0)
                            nc.tensor.matmul(
                                lbc_ps[:, col0:], lhsT=ones_mat,
                                rhs=pT_cur[:, col0:],
                                start=(kt == 0), stop=(kt == nkt - 1))
                            nc.tensor.matmul(
                                oT[:, col0:],
                                lhsT=v_sb[:, b * (S // 128) + kt,
                                          h * 128:(h + 1) * 128],
                                rhs=pT_cur[:, col0:],
                                start=(kt == 0), stop=(kt == nkt - 1))
                        # emit the previous chunk's remaining fin copies
                        # before the reciprocal chain so the pproj drain
                        # isn't queued behind it on the DVE
                        if h == 0 and pending_wo is not None:
                            wo_phase(*pending_wo, sts=(2, 3), emit_dma=True)
                            pending_wo = None
                        # free the l PSUM bank promptly via an ACT copy
                        # (the DVE queue may be clogged by fin copies),
                        # then reciprocal + normalize from SBUF
                        l_sb = bcp.tile([128, SC], f32, tag="lsb")
                        nc.scalar.copy(out=l_sb, in_=lbc_ps)
                        rbc = bcp.tile([128, SC], f32, tag="rbc")
                        nc.vector.reciprocal_approx_fast(out=rbc, in_=l_sb)
                        nc.vector.tensor_mul(on_sb[:, h, :], oT, rbc)

                    pending_wo = (on_sb, s0g, stage)
            wo_phase(*pending_wo, emit_dma=True, dma_per_st=True)
    nc.compile()
    return nc


def _prep_inputs(x, freqs_cis, wq, wk, wv, wo):
    x = np.asarray(x, dtype=np.float32)
    freqs = np.asarray(freqs_cis, dtype=np.float32)
    wq = np.asarray(wq, dtype=np.float32)
    wk = np.asarray(wk, dtype=np.float32)
    wv = np.asarray(wv, dtype=np.float32)
    wo = np.asarray(wo, dtype=np.float32)

    xT = x.reshape(B * S, HID).T.astype(F16, order="C")
    cos_e = np.ascontiguousarray(np.repeat(freqs[:, :, 0].T, 2, axis=0),
                                 dtype=np.float32)
    sin_e = np.ascontiguousarray(np.repeat(freqs[:, :, 1].T, 2, axis=0),
                                 dtype=np.float32)
    psgn = np.zeros((128, 128), F16)
    idx = np.arange(64)
    psgn[2 * idx, 2 * idx + 1] = 1.0
    psgn[2 * idx + 1, 2 * idx] = -1.0
    # causal bias, passed pre-transposed for use as matmul lhsT:
    # bias[kp, qf] = -60000 where kp > qf (future key), else 0
    kp = np.arange(128)[:, None]
    qf = np.arange(128)[None, :]
    tri = np.ascontiguousarray(np.where(kp > qf, -60000.0, 0.0).T
                               ).astype(F16)
    ident = np.eye(128, dtype=F16)

    in_maps = []
    for c in range(NCORES):
        sl = slice(DC * c, DC * (c + 1))
        wqT = wq[sl, :].T.astype(F16, order="C")
        in_maps.append({
            "xT": xT,
            "wqTa": np.ascontiguousarray(wqT[:, :128]),
            "wqTb": np.ascontiguousarray(wqT[:, 128:]),
            "wkT": wk[sl, :].T.astype(F16, order="C"),
            "wvT": wv[sl, :].T.astype(F16, order="C"),
            "woT": wo[:, sl].T.astype(F16, order="C"),
            "cose": cos_e,
            "sine": sin_e,
            "psgn": psgn,
            "tri": tri,
            "ident": ident,
        })
    return in_maps


def kernel(x, freqs_cis, wq, wk, wv, wo):
    global LAST_RESULT
    _register_ntff_hook()
    from concourse import bass_utils

    if "nc" not in _CACHE:
        _CACHE["nc"] = _build()
    nc = _CACHE["nc"]

    in_maps = _prep_inputs(x, freqs_cis, wq, wk, wv, wo)
    res = bass_utils.run_bass_kernel_spmd(
        nc, in_maps, core_ids=list(range(NCORES)))
    LAST_RESULT = res
    acc = np.zeros((B * S, HID), np.float64)
    for r in res.results:
        acc += r["out"].astype(np.float64)
    return acc.reshape(B, S, HID).astype(np.float32)


# revision 16
# speedup vs baseline: 1.2987x; 1.0102x over previous
"""Causal attention (B=2, S=2048, HID=2048, H=16, D=128) on 8 NeuronCores.

Sharding: tensor-parallel over heads — core c owns heads (2c, 2c+1).
Each core: projects Q/K/V for its heads (fp16 matmuls, fp32 PSUM accum),
applies rope (sign-permutation matmul + cos/sin DVE combine), computes
causal attention with transposed scores (k on partitions, q on free) so
softmax needs no transposes, then computes its partial contribution to
the output projection (contraction over its 256 hid columns of wo).
Host sums the 8 fp16 partial outputs.

All dram tensors carrying weights/x are pre-arranged host-side into the
exact SBUF image ([partition, ...] contiguous) so every DMA moves 4-16KB
per partition in 128 descriptors instead of thousands of 256-512B ones.

Chunk schedule: batch-1 chunk-0's attention is deferred to the very end
(its projections run in normal order).  The tail of the kernel is then a
4-tile attention instead of a 16-tile one, which shortens the exposed
softmax->normalize->wo drain after the last overlappable work.

Softmax uses exp without max subtraction: scores are ~N(0,1) after the
1/sqrt(D) scale (max |score| ~ 7), safe in fp32/fp16.
"""
import math
import sys
import types

import numpy as np

B, S, HID, H = 2, 2048, 2048, 16
D = 128
NCORES = 8
HPC = H // NCORES            # heads per core
DC = HPC * D                 # hid slice per core
SC = 512                     # seq chunk
NQC = S // SC                # chunks per batch
NHT = HID // 128             # hid tiles
CH = B * NQC                 # total chunks
F16 = np.float16


def _register_ntff_hook():
    """The agent image's antenv lacks axon_hooks; recreate it so
    run_bass_kernel_spmd(trace=True) can capture NTFF profiles."""
    try:
        from antenv.axon_hooks import get_axon_ntff_profile_hook  # noqa: F401
        return
    except ImportError:
        pass
    try:
        import antenv
        from trn_agent_boot.trn_boot import _ntff_profile_via_ctypes
        mod = types.ModuleType("antenv.axon_hooks")
        _hook = [None]
        mod.set_axon_ntff_profile_hook = lambda h: _hook.__setitem__(0, h)
        mod.get_axon_ntff_profile_hook = lambda: _hook[0]
        sys.modules["antenv.axon_hooks"] = mod
        antenv.axon_hooks = mod
        mod.set_axon_ntff_profile_hook(
            _ntff_profile_via_ctypes("/opt/axon/libaxon_pjrt.so"))
    except Exception:
        pass


_CACHE = {}
LAST_RESULT = None


def _build():
    import concourse.bass as bass  # noqa: F401
    import concourse.mybir as mybir
    import concourse.tile as tile
    from concourse import bacc

    f16 = mybir.dt.float16
    f32 = mybir.dt.float32
    EXP = mybir.ActivationFunctionType.Exp
    ISCALE = float(1.0 / math.sqrt(D))

    nc = bacc.Bacc("TRN2", target_bir_lowering=False, debug=False,
                   num_devices=NCORES)

    xP = nc.dram_tensor("xP", [128, CH * NHT * SC], f16,
                        kind="ExternalInput").ap()
    wqPa = nc.dram_tensor("wqPa", [128, NHT * 128], f16,
                          kind="ExternalInput").ap()
    wqPb = nc.dram_tensor("wqPb", [128, NHT * 128], f16,
                          kind="ExternalInput").ap()
    wkP = nc.dram_tensor("wkP", [128, NHT * DC], f16,
                         kind="ExternalInput").ap()
    wvP = nc.dram_tensor("wvP", [128, NHT * DC], f16,
                         kind="ExternalInput").ap()
    woP = nc.dram_tensor("woP", [128, HPC * HID], f16,
                         kind="ExternalInput").ap()
    cose = nc.dram_tensor("cose", [128, S], f32, kind="ExternalInput").ap()
    sine = nc.dram_tensor("sine", [128, S], f32, kind="ExternalInput").ap()
    psgn = nc.dram_tensor("psgn", [128, 128], f16, kind="ExternalInput").ap()
    tri = nc.dram_tensor("tri", [128, 128], f16, kind="ExternalInput").ap()
    ident = nc.dram_tensor("ident", [128, 128], f16, kind="ExternalInput").ap()
    out = nc.dram_tensor("out", [B * S, HID], f16, kind="ExternalOutput").ap()

    with tile.TileContext(nc) as tc:
        with tc.tile_pool(name="consts", bufs=1) as consts, \
             tc.tile_pool(name="kv", bufs=1) as kv, \
             tc.tile_pool(name="xtp", bufs=2) as xtp, \
             tc.tile_pool(name="qdp", bufs=2) as qdp, \
             tc.tile_pool(name="ropep", bufs=4) as ropep, \
             tc.tile_pool(name="ptp", bufs=4) as ptp, \
             tc.tile_pool(name="onp", bufs=2) as onp, \
             tc.tile_pool(name="bcp", bufs=2) as bcp, \
             tc.tile_pool(name="stgp", bufs=2) as stgp, \
             tc.tile_pool(name="pstream", bufs=2, space="PSUM") as pstream, \
             tc.tile_pool(name="pacc", bufs=2, space="PSUM") as pacc, \
             tc.tile_pool(name="pproj", bufs=3, space="PSUM") as pproj, \
             tc.tile_pool(name="lps", bufs=1, space="PSUM") as lps:

            # ---- constants ----
            # small psgn/tri first so PE warmup can start; everything else
            # streams on the sync queue behind the first chunk's critical
            # loads (xta+wqa), ordered by first-consumption time.  The
            # gpsimd queue is kept clear for x chunk streaming.
            psgn_sb = consts.tile([128, 128], f16)
            tri_sb = consts.tile([128, 128], f16)
            ident_sb = consts.tile([128, 128], f16)
            nc.sync.dma_start(out=psgn_sb, in_=psgn[:, :])
            nc.sync.dma_start(out=tri_sb, in_=tri[:, :])
            nc.sync.dma_start(out=ident_sb, in_=ident[:, :])
            # warm up the PE (HAM clock gate) while the first DMAs land
            warm_ps = pstream.tile([128, 128], f32, tag="st")
            for _ in range(48):
                nc.tensor.matmul(warm_ps, lhsT=psgn_sb, rhs=tri_sb,
                                 start=True, stop=True)
            wqa_sb = consts.tile([128, NHT, 128], f16)
            wqb_sb = consts.tile([128, NHT, 128], f16)
            wk_sb = consts.tile([128, NHT, DC], f16)
            wv_sb = consts.tile([128, NHT, DC], f16)
            cos_sb = consts.tile([128, S], f32)
            sin_sb = consts.tile([128, S], f32)
            ones_mat = consts.tile([128, 128], f16)
            nc.vector.memset(ones_mat, 1.0)
            wo_sb = consts.tile([128, HPC, HID], f16)

            def load_late_consts():
                # issued after the first chunk's xta; all on the sync queue
                # so they cannot overtake the critical first loads
                nc.sync.dma_start(out=wqa_sb, in_=wqPa[:, :])
                nc.sync.dma_start(out=wqb_sb, in_=wqPb[:, :])
                nc.sync.dma_start(out=cos_sb, in_=cose[:, :])
                nc.sync.dma_start(out=wk_sb, in_=wkP[:, :])
                nc.sync.dma_start(out=sin_sb, in_=sine[:, :])
                nc.sync.dma_start(out=wv_sb, in_=wvP[:, :])
                nc.sync.dma_start(out=wo_sb, in_=woP[:, :])

            # ---- persistent K/V for the core's heads ----
            kd_sb = kv.tile([128, HPC, B * S], f16)     # [d, head, b*2048+s]
            v_sb = kv.tile([128, B * S // 128, DC], f16)  # [s%128, stile, (h,d)]

            def wo_phase(on_sb, s0g, stage, sts=(0, 1, 2, 3),
                         emit_dma=False, dma_per_st=False, use_act=False):
                # ---------- output projection for a finished chunk ----------
                # results staged in SBUF (fp16 partials) and shipped with a
                # single DMA per chunk (or per st-row for the final chunk so
                # the last transfer isn't one big serial tail); sub-phases
                # are interleaved into the next chunk's attention.  use_act
                # moves the PSUM drain to the scalar engine (for the tail,
                # where the DVE cast cadence would pace the PE).
                for st in sts:
                    for ec in range(HID // 512):
                        fin_ps = pproj.tile([128, 512], f32, tag="pj")
                        for dt in range(HPC):
                            nc.tensor.matmul(
                                fin_ps,
                                lhsT=on_sb[:, dt, st * 128:(st + 1) * 128],
                                rhs=wo_sb[:, dt, ec * 512:(ec + 1) * 512],
                                start=(dt == 0), stop=(dt == HPC - 1))
                        dst = stage[:, st, ec * 512:(ec + 1) * 512]
                        if use_act:
                            nc.scalar.copy(out=dst, in_=fin_ps)
                        else:
                            nc.vector.tensor_copy(out=dst, in_=fin_ps)
                    if dma_per_st:
                        nc.sync.dma_start(
                            out=out[s0g + st * 128:s0g + (st + 1) * 128, :],
                            in_=stage[:, st, :])
                if emit_dma and not dma_per_st:
                    nc.sync.dma_start(
                        out=out[s0g:s0g + SC, :].rearrange(
                            "(st p) e -> p st e", p=128),
                        in_=stage)

            # pending output projection of the previous chunk:
            # [on_sb, s0g, stage, first_half_emitted]
            pending_wo = [None]

            def wo_pending_first_half():
                if pending_wo[0] is not None and not pending_wo[0][3]:
                    wo_phase(*pending_wo[0][:3], sts=(0, 1))
                    pending_wo[0][3] = True

            def wo_pending_finish():
                if pending_wo[0] is not None:
                    sts = (2, 3) if pending_wo[0][3] else (0, 1, 2, 3)
                    wo_phase(*pending_wo[0][:3], sts=sts, emit_dma=True)
                    pending_wo[0] = None

            def chunk_proj(b, qc, qd_tag):
                s0g = b * S + qc * SC
                q0 = qc * SC
                c = b * NQC + qc
                # two separate half-tiles so dependency tracking lets
                # ht 0-7 matmuls start before the second half lands;
                # first chunk puts xta on the sync queue ahead of the
                # weight stream
                half = NHT // 2
                xta = xtp.tile([128, half, SC], f16, tag="xta")
                xtb = xtp.tile([128, half, SC], f16, tag="xtb")
                dma_a = nc.sync.dma_start if s0g == 0 \
                    else nc.gpsimd.dma_start
                base = c * NHT * SC
                dma_a(out=xta, in_=xP[:, base:base + half * SC])
                nc.gpsimd.dma_start(
                    out=xtb, in_=xP[:, base + half * SC:base + NHT * SC])
                if s0g == 0:
                    load_late_consts()

                def xt_sl(ht, cols=slice(None)):
                    t = xta if ht < half else xtb
                    return t[:, ht % half, cols]

                qd_c = qdp.tile([128, HPC, SC], f16, tag=qd_tag)

                def emit_rope(acc, qraw, dest):
                    # dest = acc*cos + (psgn.T@acc)*sin — the u matmul
                    # is emitted one projection group late so the PE
                    # doesn't stall on the qraw ACT copy
                    u_ps = pstream.tile([128, SC], f32, tag="st")
                    nc.tensor.matmul(u_ps, lhsT=psgn_sb, rhs=qraw,
                                     start=True, stop=True)
                    t0 = ropep.tile([128, SC], f16, tag="t0")
                    nc.vector.tensor_mul(t0, acc, cos_sb[:, q0:q0 + SC])
                    t1 = ropep.tile([128, SC], f16, tag="t1")
                    nc.vector.tensor_mul(t1, u_ps, sin_sb[:, q0:q0 + SC])
                    nc.vector.tensor_add(dest, t0, t1)

                rope_pending = None
                for which in ("q", "k"):
                    for dt in range(HPC):
                        if which == "q":
                            w_sl = (wqa_sb if dt == 0 else wqb_sb)
                        for ht in range(NHT):
                            if which == "q":
                                lhsT = w_sl[:, ht, :]
                            else:
                                lhsT = wk_sb[:, ht,
                                             dt * 128:(dt + 1) * 128]
                            if ht == 0:
                                acc = pproj.tile([128, SC], f32, tag="pj")
                            nc.tensor.matmul(
                                acc, lhsT=lhsT, rhs=xt_sl(ht),
                                start=(ht == 0), stop=(ht == NHT - 1))
                        qraw = ropep.tile([128, SC], f16, tag="qraw")
                        nc.scalar.copy(out=qraw, in_=acc)
                        if rope_pending is not None:
                            emit_rope(*rope_pending)
                        if which == "q":
                            dest = qd_c[:, dt, :]
                        else:
                            dest = kd_sb[:, dt, s0g:s0g + SC]
                        rope_pending = (acc, qraw, dest)
                for st in range(SC // 128):
                    vacc = pproj.tile([128, DC], f32, tag="pj")
                    for ht in range(NHT):
                        nc.tensor.matmul(
                            vacc,
                            lhsT=xt_sl(ht, slice(st * 128, (st + 1) * 128)),
                            rhs=wv_sb[:, ht, :],
                            start=(ht == 0), stop=(ht == NHT - 1))
                    if rope_pending is not None:
                        emit_rope(*rope_pending)
                        rope_pending = None
                    nc.vector.tensor_copy(
                        out=v_sb[:, (s0g // 128) + st, :], in_=vacc)
                return qd_c

            def chunk_attn(b, qc, qd_c):
                s0g = b * S + qc * SC
                q0 = qc * SC
                on_sb = onp.tile([128, HPC, SC], f16, tag="on")
                stage = stgp.tile([128, SC // 128, HID], f16, tag="stg")
                for h in range(HPC):
                    oT = pacc.tile([128, SC], f32, tag="acc")
                    # column sums of probs, broadcast to all 128
                    # partitions by an all-ones stationary matrix
                    lbc_ps = lps.tile([128, SC], f32, tag="l")
                    nkt = qc * 4 + 4

                    def emit_probs(kt):
                        # scores matmul + exp; on the diagonal tile a
                        # second tiny matmul accumulates a -60000
                        # upper-triangle bias (tri_sb.T @ I) so exp
                        # gives exact causal zeros — no vector-engine
                        # masking in the dependency chain
                        col0 = max(0, 128 * kt - q0)
                        diag = kt >= qc * 4
                        sp = pstream.tile([128, SC], f32, tag="st")
                        nc.tensor.matmul(
                            sp[:, col0:],
                            lhsT=kd_sb[:, h,
                                       b * S + kt * 128:
                                       b * S + (kt + 1) * 128],
                            rhs=qd_c[:, h, col0:],
                            start=True, stop=not diag)
                        if diag:
                            j = 128 * (kt - qc * 4)
                            nc.tensor.matmul(
                                sp[:, j:j + 128], lhsT=tri_sb,
                                rhs=ident_sb, start=False, stop=True)
                        pT = ptp.tile([128, SC], f16, tag="pt")
                        nc.scalar.activation(out=pT[:, col0:],
                                             in_=sp[:, col0:],
                                             func=EXP, scale=ISCALE)
                        return pT

                    # software-pipelined (depth 2): scores for kt+1/kt+2
                    # are emitted before the l/PV matmuls of kt, so the
                    # PE has work while the exp for kt runs on ACT
                    pts = [emit_probs(kt) for kt in range(min(2, nkt))]
                    for kt in range(nkt):
                        col0 = max(0, 128 * kt - q0)
                        if kt + 2 < nkt:
                            pts.append(emit_probs(kt + 2))
                        pT_cur = pts.pop(0)
                        nc.tensor.matmul(
                            lbc_ps[:, col0:], lhsT=ones_mat,
                            rhs=pT_cur[:, col0:],
                            start=(kt == 0), stop=(kt == nkt - 1))
                        nc.tensor.matmul(
                            oT[:, col0:],
                            lhsT=v_sb[:, b * (S // 128) + kt,
                                      h * 128:(h + 1) * 128],
                            rhs=pT_cur[:, col0:],
                            start=(kt == 0), stop=(kt == nkt - 1))
                    # emit the previous chunk's remaining fin copies
                    # before the reciprocal chain so the pproj drain
                    # isn't queued behind it on the DVE
                    if h == 0:
                        wo_pending_finish()
                    # free the l PSUM bank promptly via an ACT copy
                    # (the DVE queue may be clogged by fin copies),
                    # then reciprocal + normalize from SBUF
                    l_sb = bcp.tile([128, SC], f32, tag="lsb")
                    nc.scalar.copy(out=l_sb, in_=lbc_ps)
                    rbc = bcp.tile([128, SC], f32, tag="rbc")
                    nc.vector.reciprocal_approx_fast(out=rbc, in_=l_sb)
                    nc.vector.tensor_mul(on_sb[:, h, :], oT, rbc)
                pending_wo[0] = [on_sb, s0g, stage, False]

            # ---- chunk schedule ----
            # (b, qc, 'PA' | 'P' | 'A'): batch-1 chunk-0's attention is
            # deferred to the very end so the exposed tail is a 4-tile
            # attention instead of a 16-tile one
            schedule = [(0, 0, "PA"), (0, 1, "PA"), (0, 2, "PA"),
                        (0, 3, "PA"), (1, 0, "P"), (1, 1, "PA"),
                        (1, 2, "PA"), (1, 3, "PA"), (1, 0, "A")]
            qd_saved = {}
            for b, qc, mode in schedule:
                if "P" in mode:
                    tag = "qdD" if mode == "P" else "qd"
                    qd_c = chunk_proj(b, qc, tag)
                    qd_saved[(b, qc)] = qd_c
                    wo_pending_first_half()
                if "A" in mode:
                    if mode == "A":
                        wo_pending_first_half()
                    chunk_attn(b, qc, qd_saved.pop((b, qc)))
            wo_phase(*pending_wo[0][:3], emit_dma=True, dma_per_st=True,
                     use_act=True)
    nc.compile()
    return nc


def _prep_inputs(x, freqs_cis, wq, wk, wv, wo):
    x = np.asarray(x, dtype=np.float32)
    freqs = np.asarray(freqs_cis, dtype=np.float32)
    wq = np.asarray(wq, dtype=np.float32)
    wk = np.asarray(wk, dtype=np.float32)
    wv = np.asarray(wv, dtype=np.float32)
    wo = np.asarray(wo, dtype=np.float32)

    # x as the SBUF image: [p, chunk, ht, s] contiguous
    xT = x.reshape(B * S, HID).T.astype(F16)               # [(ht p), (c s)]
    xP = np.ascontiguousarray(
        xT.reshape(NHT, 128, CH, SC).transpose(1, 2, 0, 3)).reshape(128, -1)
    cos_e = np.ascontiguousarray(np.repeat(freqs[:, :, 0].T, 2, axis=0),
                                 dtype=np.float32)
    sin_e = np.ascontiguousarray(np.repeat(freqs[:, :, 1].T, 2, axis=0),
                                 dtype=np.float32)
    psgn = np.zeros((128, 128), F16)
    idx = np.arange(64)
    psgn[2 * idx, 2 * idx + 1] = 1.0
    psgn[2 * idx + 1, 2 * idx] = -1.0
    # causal bias, passed pre-transposed for use as matmul lhsT:
    # bias[kp, qf] = -60000 where kp > qf (future key), else 0
    kp = np.arange(128)[:, None]
    qf = np.arange(128)[None, :]
    tri = np.ascontiguousarray(np.where(kp > qf, -60000.0, 0.0).T
                               ).astype(F16)
    ident = np.eye(128, dtype=F16)

    def wimg(wT, dcols):
        # [(ht p), d] -> [p, ht, d] SBUF image, flattened
        nht = wT.shape[0] // 128
        return np.ascontiguousarray(
            wT.reshape(nht, 128, dcols).transpose(1, 0, 2)).reshape(128, -1)

    in_maps = []
    for c in range(NCORES):
        sl = slice(DC * c, DC * (c + 1))
        wqT = wq[sl, :].T.astype(F16)                      # [2048, 256]
        in_maps.append({
            "xP": xP,
            "wqPa": wimg(np.ascontiguousarray(wqT[:, :128]), 128),
            "wqPb": wimg(np.ascontiguousarray(wqT[:, 128:]), 128),
            "wkP": wimg(wk[sl, :].T.astype(F16), DC),
            "wvP": wimg(wv[sl, :].T.astype(F16), DC),
            "woP": wimg(wo[:, sl].T.astype(F16), HID),
            "cose": cos_e,
            "sine": sin_e,
            "psgn": psgn,
            "tri": tri,
            "ident": ident,
        })
    return in_maps


def kernel(x, freqs_cis, wq, wk, wv, wo):
    global LAST_RESULT
    _register_ntff_hook()
    from concourse import bass_utils

    if "nc" not in _CACHE:
        _CACHE["nc"] = _build()
    nc = _CACHE["nc"]

    in_maps = _prep_inputs(x, freqs_cis, wq, wk, wv, wo)
    res = bass_utils.run_bass_kernel_spmd(
        nc, in_maps, core_ids=list(range(NCORES)))
    LAST_RESULT = res
    acc = np.zeros((B * S, HID), np.float64)
    for r in res.results:
        acc += r["out"].astype(np.float64)
    return acc.reshape(B, S, HID).astype(np.float32)


# revision 24
# speedup vs baseline: 1.3187x; 1.0154x over previous
"""Causal attention (B=2, S=2048, HID=2048, H=16, D=128) on 8 NeuronCores.

Sharding: tensor-parallel over heads — core c owns heads (2c, 2c+1).
Each core: projects Q/K/V for its heads (fp16 matmuls, fp32 PSUM accum),
applies rope (sign-permutation matmul + cos/sin DVE combine), computes
causal attention with transposed scores (k on partitions, q on free) so
softmax needs no transposes, then computes its partial contribution to
the output projection (contraction over its 256 hid columns of wo).
Host sums the 8 fp16 partial outputs.

All dram tensors carrying weights/x are pre-arranged host-side into the
exact SBUF image ([partition, ...] contiguous) so every DMA moves 4-16KB
per partition in 128 descriptors instead of thousands of 256-512B ones.

Chunk schedule: batch-1 chunk-0's attention is deferred to the very end
(its projections run in normal order).  The tail of the kernel is then a
4-tile attention instead of a 16-tile one, which shortens the exposed
softmax->normalize->wo drain after the last overlappable work.

Softmax uses exp without max subtraction: scores are ~N(0,1) after the
1/sqrt(D) scale (max |score| ~ 7), safe in fp32/fp16.
"""
import math
import sys
import types

import numpy as np

B, S, HID, H = 2, 2048, 2048, 16
D = 128
NCORES = 8
HPC = H // NCORES            # heads per core
DC = HPC * D                 # hid slice per core
SC = 512                     # seq chunk
NQC = S // SC                # chunks per batch
NHT = HID // 128             # hid tiles
CH = B * NQC                 # total chunks
F16 = np.float16


def _register_ntff_hook():
    """The agent image's antenv lacks axon_hooks; recreate it so
    run_bass_kernel_spmd(trace=True) can capture NTFF profiles."""
    try:
        from antenv.axon_hooks import get_axon_ntff_profile_hook  # noqa: F401
        return
    except ImportError:
        pass
    try:
        import antenv
        from trn_agent_boot.trn_boot import _ntff_profile_via_ctypes
        mod = types.ModuleType("antenv.axon_hooks")
        _hook = [None]
        mod.set_axon_ntff_profile_hook = lambda h: _hook.__setitem__(0, h)
        mod.get_axon_ntff_profile_hook = lambda: _hook[0]
        sys.modules["antenv.axon_hooks"] = mod
        antenv.axon_hooks = mod
        mod.set_axon_ntff_profile_hook(
            _ntff_profile_via_ctypes("/opt/axon/libaxon_pjrt.so"))
    except Exception:
        pass


_CACHE = {}
LAST_RESULT = None


def _build():
    import concourse.bass as bass  # noqa: F401
    import concourse.mybir as mybir
    import concourse.tile as tile
    from concourse import bacc

    f16 = mybir.dt.float16
    f32 = mybir.dt.float32
    EXP = mybir.ActivationFunctionType.Exp
    ISCALE = float(1.0 / math.sqrt(D))

    nc = bacc.Bacc("TRN2", target_bir_lowering=False, debug=False,
                   num_devices=NCORES)

    xP = nc.dram_tensor("xP", [128, CH * NHT * SC], f16,
                        kind="ExternalInput").ap()
    wqPa = nc.dram_tensor("wqPa", [128, NHT * 128], f16,
                          kind="ExternalInput").ap()
    wqPb = nc.dram_tensor("wqPb", [128, NHT * 128], f16,
                          kind="ExternalInput").ap()
    wkP = nc.dram_tensor("wkP", [128, NHT * DC], f16,
                         kind="ExternalInput").ap()
    wvP = nc.dram_tensor("wvP", [128, NHT * DC], f16,
                         kind="ExternalInput").ap()
    woP = nc.dram_tensor("woP", [128, HPC * HID], f16,
                         kind="ExternalInput").ap()
    cose = nc.dram_tensor("cose", [128, S], f32, kind="ExternalInput").ap()
    sine = nc.dram_tensor("sine", [128, S], f32, kind="ExternalInput").ap()
    # psgn | tri | ident stacked into one small DMA
    c3 = nc.dram_tensor("c3", [128, 3 * 128], f16, kind="ExternalInput").ap()
    out = nc.dram_tensor("out", [B * S, HID], f16, kind="ExternalOutput").ap()

    with tile.TileContext(nc) as tc:
        with tc.tile_pool(name="consts", bufs=1) as consts, \
             tc.tile_pool(name="kv", bufs=1) as kv, \
             tc.tile_pool(name="xtp", bufs=2) as xtp, \
             tc.tile_pool(name="qdp", bufs=2) as qdp, \
             tc.tile_pool(name="ropep", bufs=4) as ropep, \
             tc.tile_pool(name="ptp", bufs=4) as ptp, \
             tc.tile_pool(name="onp", bufs=2) as onp, \
             tc.tile_pool(name="bcp", bufs=2) as bcp, \
             tc.tile_pool(name="stgp", bufs=2) as stgp, \
             tc.tile_pool(name="pstream", bufs=2, space="PSUM") as pstream, \
             tc.tile_pool(name="pacc", bufs=2, space="PSUM") as pacc, \
             tc.tile_pool(name="pproj", bufs=3, space="PSUM") as pproj, \
             tc.tile_pool(name="lps", bufs=1, space="PSUM") as lps:

            # ---- constants ----
            # one tiny combined DMA first so PE warmup can start right
            # away; everything else streams on the sync queue behind the
            # first chunk's critical loads (xta+wqa), ordered by
            # first-consumption time.  Nearly all DMA goes through the
            # sync queue: a second busy queue steals HBM bandwidth from
            # the critical path (queues are serviced round-robin).
            c3_sb = consts.tile([128, 3, 128], f16)
            nc.sync.dma_start(out=c3_sb, in_=c3[:, :])
            psgn_sb = c3_sb[:, 0, :]
            tri_sb = c3_sb[:, 1, :]
            ident_sb = c3_sb[:, 2, :]
            # warm up the PE (HAM clock gate) while the first DMAs land
            warm_ps = pstream.tile([128, 128], f32, tag="st")
            for _ in range(64):
                nc.tensor.matmul(warm_ps, lhsT=psgn_sb, rhs=tri_sb,
                                 start=True, stop=True)
            wqa_sb = consts.tile([128, NHT, 128], f16)
            wqb_sb = consts.tile([128, NHT, 128], f16)
            wk_sb = consts.tile([128, NHT, DC], f16)
            wv_sb = consts.tile([128, NHT, DC], f16)
            cos_sb = consts.tile([128, S], f32)
            sin_sb = consts.tile([128, S], f32)
            ones_mat = consts.tile([128, 128], f16)
            nc.vector.memset(ones_mat, 1.0)
            wo_sb = consts.tile([128, HPC, HID], f16)

            def load_late_consts():
                # issued after the first chunk's xta; all on the sync queue
                # so they cannot overtake the critical first loads
                nc.sync.dma_start(out=wqa_sb, in_=wqPa[:, :])
                nc.sync.dma_start(out=wqb_sb, in_=wqPb[:, :])
                nc.sync.dma_start(out=cos_sb, in_=cose[:, :])
                nc.sync.dma_start(out=wk_sb, in_=wkP[:, :])
                nc.sync.dma_start(out=sin_sb, in_=sine[:, :])
                nc.sync.dma_start(out=wv_sb, in_=wvP[:, :])
                nc.sync.dma_start(out=wo_sb, in_=woP[:, :])

            # ---- persistent K/V for the core's heads ----
            kd_sb = kv.tile([128, HPC, B * S], f16)     # [d, head, b*2048+s]
            v_sb = kv.tile([128, B * S // 128, DC], f16)  # [s%128, stile, (h,d)]

            def wo_phase(on_sb, s0g, stage, sts=(0, 1, 2, 3),
                         emit_dma=False, dma_per_st=False, use_act=False):
                # ---------- output projection for a finished chunk ----------
                # results staged in SBUF (fp16 partials) and shipped with a
                # single DMA per chunk (or per st-row for the final chunk so
                # the last transfer isn't one big serial tail); sub-phases
                # are interleaved into the next chunk's attention.  use_act
                # moves the PSUM drain to the scalar engine (for the tail,
                # where the DVE cast cadence would pace the PE).
                for st in sts:
                    for ec in range(HID // 512):
                        fin_ps = pproj.tile([128, 512], f32, tag="pj")
                        for dt in range(HPC):
                            nc.tensor.matmul(
                                fin_ps,
                                lhsT=on_sb[:, dt, st * 128:(st + 1) * 128],
                                rhs=wo_sb[:, dt, ec * 512:(ec + 1) * 512],
                                start=(dt == 0), stop=(dt == HPC - 1))
                        dst = stage[:, st, ec * 512:(ec + 1) * 512]
                        if use_act:
                            nc.scalar.copy(out=dst, in_=fin_ps)
                        else:
                            nc.vector.tensor_copy(out=dst, in_=fin_ps)
                    if dma_per_st:
                        nc.sync.dma_start(
                            out=out[s0g + st * 128:s0g + (st + 1) * 128, :],
                            in_=stage[:, st, :])
                if emit_dma and not dma_per_st:
                    nc.sync.dma_start(
                        out=out[s0g:s0g + SC, :].rearrange(
                            "(st p) e -> p st e", p=128),
                        in_=stage)

            # pending output projection of the previous chunk:
            # [on_sb, s0g, stage, first_half_emitted]
            pending_wo = [None]

            def wo_pending_first_half():
                if pending_wo[0] is not None and not pending_wo[0][3]:
                    wo_phase(*pending_wo[0][:3], sts=(0, 1))
                    pending_wo[0][3] = True

            def wo_pending_finish():
                if pending_wo[0] is not None:
                    sts = (2, 3) if pending_wo[0][3] else (0, 1, 2, 3)
                    wo_phase(*pending_wo[0][:3], sts=sts, emit_dma=True)
                    pending_wo[0] = None

            def chunk_proj(b, qc, qd_tag):
                s0g = b * S + qc * SC
                q0 = qc * SC
                c = b * NQC + qc
                # two separate half-tiles so dependency tracking lets
                # ht 0-7 matmuls start before the second half lands;
                # first chunk puts xta on the sync queue ahead of the
                # weight stream
                half = NHT // 2
                xta = xtp.tile([128, half, SC], f16, tag="xta")
                xtb = xtp.tile([128, half, SC], f16, tag="xtb")
                # chunk 0: xta on sync, xtb concurrently on gpsimd, then
                # the whole weight stream behind them on sync.  Later
                # chunks: everything on sync, which is drained by then —
                # keeping the second queue idle maximizes critical-path
                # bandwidth.
                dma_b = nc.gpsimd.dma_start if s0g == 0 \
                    else nc.sync.dma_start
                base = c * NHT * SC
                nc.sync.dma_start(out=xta, in_=xP[:, base:base + half * SC])
                dma_b(out=xtb, in_=xP[:, base + half * SC:base + NHT * SC])
                if s0g == 0:
                    load_late_consts()

                def xt_sl(ht, cols=slice(None)):
                    t = xta if ht < half else xtb
                    return t[:, ht % half, cols]

                qd_c = qdp.tile([128, HPC, SC], f16, tag=qd_tag)

                def emit_rope(acc, qraw, dest):
                    # dest = acc*cos + (psgn.T@acc)*sin — the u matmul
                    # is emitted one projection group late so the PE
                    # doesn't stall on the qraw ACT copy
                    u_ps = pstream.tile([128, SC], f32, tag="st")
                    nc.tensor.matmul(u_ps, lhsT=psgn_sb, rhs=qraw,
                                     start=True, stop=True)
                    t0 = ropep.tile([128, SC], f16, tag="t0")
                    nc.vector.tensor_mul(t0, acc, cos_sb[:, q0:q0 + SC])
                    t1 = ropep.tile([128, SC], f16, tag="t1")
                    nc.vector.tensor_mul(t1, u_ps, sin_sb[:, q0:q0 + SC])
                    nc.vector.tensor_add(dest, t0, t1)

                rope_pending = None
                for which in ("q", "k"):
                    for dt in range(HPC):
                        if which == "q":
                            w_sl = (wqa_sb if dt == 0 else wqb_sb)
                        for ht in range(NHT):
                            if which == "q":
                                lhsT = w_sl[:, ht, :]
                            else:
                                lhsT = wk_sb[:, ht,
                                             dt * 128:(dt + 1) * 128]
                            if ht == 0:
                                acc = pproj.tile([128, SC], f32, tag="pj")
                            nc.tensor.matmul(
                                acc, lhsT=lhsT, rhs=xt_sl(ht),
                                start=(ht == 0), stop=(ht == NHT - 1))
                        qraw = ropep.tile([128, SC], f16, tag="qraw")
                        nc.scalar.copy(out=qraw, in_=acc)
                        if rope_pending is not None:
                            emit_rope(*rope_pending)
                        if which == "q":
                            dest = qd_c[:, dt, :]
                        else:
                            dest = kd_sb[:, dt, s0g:s0g + SC]
                        rope_pending = (acc, qraw, dest)
                for st in range(SC // 128):
                    vacc = pproj.tile([128, DC], f32, tag="pj")
                    for ht in range(NHT):
                        nc.tensor.matmul(
                            vacc,
                            lhsT=xt_sl(ht, slice(st * 128, (st + 1) * 128)),
                            rhs=wv_sb[:, ht, :],
                            start=(ht == 0), stop=(ht == NHT - 1))
                    if rope_pending is not None:
                        emit_rope(*rope_pending)
                        rope_pending = None
                    nc.vector.tensor_copy(
                        out=v_sb[:, (s0g // 128) + st, :], in_=vacc)
                return qd_c

            def chunk_attn(b, qc, qd_c, finish_h=0):
                s0g = b * S + qc * SC
                q0 = qc * SC
                on_sb = onp.tile([128, HPC, SC], f16, tag="on")
                stage = stgp.tile([128, SC // 128, HID], f16, tag="stg")
                for h in range(HPC):
                    oT = pacc.tile([128, SC], f32, tag="acc")
                    # column sums of probs, broadcast to all 128
                    # partitions by an all-ones stationary matrix
                    lbc_ps = lps.tile([128, SC], f32, tag="l")
                    nkt = qc * 4 + 4

                    def emit_probs(kt):
                        # scores matmul + exp; on the diagonal tile a
                        # second tiny matmul accumulates a -60000
                        # upper-triangle bias (tri_sb.T @ I) so exp
                        # gives exact causal zeros — no vector-engine
                        # masking in the dependency chain
                        col0 = max(0, 128 * kt - q0)
                        diag = kt >= qc * 4
                        sp = pstream.tile([128, SC], f32, tag="st")
                        nc.tensor.matmul(
                            sp[:, col0:],
                            lhsT=kd_sb[:, h,
                                       b * S + kt * 128:
                                       b * S + (kt + 1) * 128],
                            rhs=qd_c[:, h, col0:],
                            start=True, stop=not diag)
                        if diag:
                            j = 128 * (kt - qc * 4)
                            nc.tensor.matmul(
                                sp[:, j:j + 128], lhsT=tri_sb,
                                rhs=ident_sb, start=False, stop=True)
                        pT = ptp.tile([128, SC], f16, tag="pt")
                        nc.scalar.activation(out=pT[:, col0:],
                                             in_=sp[:, col0:],
                                             func=EXP, scale=ISCALE)
                        return pT

                    # software-pipelined (depth 2): scores for kt+1/kt+2
                    # are emitted before the l/PV matmuls of kt, so the
                    # PE has work while the exp for kt runs on ACT
                    pts = [emit_probs(kt) for kt in range(min(2, nkt))]
                    for kt in range(nkt):
                        col0 = max(0, 128 * kt - q0)
                        if kt + 2 < nkt:
                            pts.append(emit_probs(kt + 2))
                        pT_cur = pts.pop(0)
                        nc.tensor.matmul(
                            lbc_ps[:, col0:], lhsT=ones_mat,
                            rhs=pT_cur[:, col0:],
                            start=(kt == 0), stop=(kt == nkt - 1))
                        nc.tensor.matmul(
                            oT[:, col0:],
                            lhsT=v_sb[:, b * (S // 128) + kt,
                                      h * 128:(h + 1) * 128],
                            rhs=pT_cur[:, col0:],
                            start=(kt == 0), stop=(kt == nkt - 1))
                    # emit the previous chunk's remaining fin copies
                    # before the reciprocal chain so the pproj drain
                    # isn't queued behind it on the DVE (for the final
                    # chunk, at h1 so those matmuls fill the last
                    # reciprocal-chain bubble)
                    if h == finish_h:
                        wo_pending_finish()
                    # free the l PSUM bank promptly via an ACT copy
                    # (the DVE queue may be clogged by fin copies),
                    # then reciprocal + normalize from SBUF
                    l_sb = bcp.tile([128, SC], f32, tag="lsb")
                    nc.scalar.copy(out=l_sb, in_=lbc_ps)
                    rbc = bcp.tile([128, SC], f32, tag="rbc")
                    nc.vector.reciprocal_approx_fast(out=rbc, in_=l_sb)
                    nc.vector.tensor_mul(on_sb[:, h, :], oT, rbc)
                pending_wo[0] = [on_sb, s0g, stage, False]

            # ---- chunk schedule ----
            # (b, qc, 'PA' | 'P' | 'A'): batch-1 chunk-0's attention is
            # deferred to the very end so the exposed tail is a 4-tile
            # attention instead of a 16-tile one
            schedule = [(0, 0, "PA"), (0, 1, "PA"), (0, 2, "PA"),
                        (0, 3, "PA"), (1, 0, "P"), (1, 1, "PA"),
                        (1, 2, "PA"), (1, 3, "PA"), (1, 0, "A")]
            qd_saved = {}
            for b, qc, mode in schedule:
                if "P" in mode:
                    tag = "qdD" if mode == "P" else "qd"
                    qd_c = chunk_proj(b, qc, tag)
                    qd_saved[(b, qc)] = qd_c
                    wo_pending_first_half()
                if "A" in mode:
                    if mode == "A":
                        wo_pending_first_half()
                    chunk_attn(b, qc, qd_saved.pop((b, qc)),
                               finish_h=HPC - 1 if mode == "A" else 0)
            wo_phase(*pending_wo[0][:3], emit_dma=True, dma_per_st=True,
                     use_act=True)
    nc.compile()
    return nc


def _prep_inputs(x, freqs_cis, wq, wk, wv, wo):
    x = np.asarray(x, dtype=np.float32)
    freqs = np.asarray(freqs_cis, dtype=np.float32)
    wq = np.asarray(wq, dtype=np.float32)
    wk = np.asarray(wk, dtype=np.float32)
    wv = np.asarray(wv, dtype=np.float32)
    wo = np.asarray(wo, dtype=np.float32)

    # x as the SBUF image: [p, chunk, ht, s] contiguous
    xT = x.reshape(B * S, HID).T.astype(F16)               # [(ht p), (c s)]
    xP = np.ascontiguousarray(
        xT.reshape(NHT, 128, CH, SC).transpose(1, 2, 0, 3)).reshape(128, -1)
    cos_e = np.ascontiguousarray(np.repeat(freqs[:, :, 0].T, 2, axis=0),
                                 dtype=np.float32)
    sin_e = np.ascontiguousarray(np.repeat(freqs[:, :, 1].T, 2, axis=0),
                                 dtype=np.float32)
    psgn = np.zeros((128, 128), F16)
    idx = np.arange(64)
    psgn[2 * idx, 2 * idx + 1] = 1.0
    psgn[2 * idx + 1, 2 * idx] = -1.0
    # causal bias, passed pre-transposed for use as matmul lhsT:
    # bias[kp, qf] = -60000 where kp > qf (future key), else 0
    kp = np.arange(128)[:, None]
    qf = np.arange(128)[None, :]
    tri = np.ascontiguousarray(np.where(kp > qf, -60000.0, 0.0).T
                               ).astype(F16)
    ident = np.eye(128, dtype=F16)
    c3 = np.ascontiguousarray(
        np.stack([psgn, tri, ident], axis=1)).reshape(128, -1)

    def wimg(wT, dcols):
        # [(ht p), d] -> [p, ht, d] SBUF image, flattened
        nht = wT.shape[0] // 128
        return np.ascontiguousarray(
            wT.reshape(nht, 128, dcols).transpose(1, 0, 2)).reshape(128, -1)

    in_maps = []
    for c in range(NCORES):
        sl = slice(DC * c, DC * (c + 1))
        wqT = wq[sl, :].T.astype(F16)                      # [2048, 256]
        in_maps.append({
            "xP": xP,
            "wqPa": wimg(np.ascontiguousarray(wqT[:, :128]), 128),
            "wqPb": wimg(np.ascontiguousarray(wqT[:, 128:]), 128),
            "wkP": wimg(wk[sl, :].T.astype(F16), DC),
            "wvP": wimg(wv[sl, :].T.astype(F16), DC),
            "woP": wimg(wo[:, sl].T.astype(F16), HID),
            "cose": cos_e,
            "sine": sin_e,
            "c3": c3,
        })
    return in_maps


def kernel(x, freqs_cis, wq, wk, wv, wo):
    global LAST_RESULT
    _register_ntff_hook()
    from concourse import bass_utils

    if "nc" not in _CACHE:
        _CACHE["nc"] = _build()
    nc = _CACHE["nc"]

    in_maps = _prep_inputs(x, freqs_cis, wq, wk, wv, wo)
    res = bass_utils.run_bass_kernel_spmd(
        nc, in_maps, core_ids=list(range(NCORES)))
    LAST_RESULT = res
    acc = np.zeros((B * S, HID), np.float64)
    for r in res.results:
        acc += r["out"].astype(np.float64)
    return acc.reshape(B, S, HID).astype(np.float32)
